# revision 4
# baseline (speedup 1.0000x reference)
"""CAM (channel attention) kernel for Trainium2, 8-core data-parallel over batch.

Per batch item (one per NeuronCore):
    energy = Q @ K^T  (C x C, contract over N=4096)
    att    = softmax(max(energy) - energy) = softmax(-energy)
    out    = gamma * (att @ V) + V  computed as (gamma*att + I) @ V_bf16

Pipeline (per core; q,k,v: [C=512, N=4096] f32 in DRAM):
  - q chunks [128,1024] f32 stream on the SP HWDGE ring, k chunks on the
    gpsimd SWDGE ring; DVE casts q to bf16, ACT casts k; the scalar HWDGE
    ring xbar-transposes both into packed qT/kT tiles, one tile per n-group,
    laid out [p, ctile, jj, 128] so every xbar dst is contiguous per
    partition (fast path) and mm1 reads qT contiguously / kT via 3D AP.
  - mm1 accumulates energy in 4 PSUM banks [128c, 512d] over the 32 n-chunks,
    group by group as transposes land.
  - softmax over the free dim: DVE min, ACT exp(bias=rowmin, scale=-1) with
    fused row-sum, DVE reciprocal; gamma/rowsum folded into att scaling and
    identity added on the diagonal so mm2 directly yields gamma*(att@V)+V.
  - att transposed via PE (identity matmul) into attT (bf16).
  - v loaded as column-blocks [128,1024] f32 (SWDGE, after k), cast to bf16;
    mm2 does 4 accumulating matmuls per (ctile, 512-col chunk); epilogue is a
    plain PSUM->SBUF copy (DVE/ACT alternating); stores on the SP ring.
"""

import numpy as np

B, C, H, W = 8, 512, 64, 64
N = H * W  # 4096
P = 128
CT = C // P  # 4 c-tiles
NJ = N // P  # 32 n-chunks
NO = N // 512  # 8 output column chunks

_nc_cache: dict = {}


def _body(nc, tc, cfg):
    from contextlib import ExitStack

    import concourse.mybir as mybir
    from concourse.bass import ts
    from concourse.masks import make_identity

    cfg = cfg or {}

    dt = mybir.dt
    f32, bf16 = dt.float32, dt.bfloat16
    X = mybir.AxisListType.X

    NG = cfg.get("ng", 4)  # n-groups for the q/k front
    GJ = NJ // NG  # 128-chunks per group
    GW = GJ * P  # columns per group chunk (1024)
    VG = cfg.get("vg", 4)  # v column-groups
    VW = N // VG  # 1024
    do = lambda phase: phase not in cfg.get("skip", ())

    qa = nc.kio["q"].ap().rearrange("(a p) w -> a p w", p=P)
    ka = nc.kio["k"].ap().rearrange("(a p) w -> a p w", p=P)
    va = nc.kio["v"].ap().rearrange("(a p) w -> a p w", p=P)
    ga = nc.kio["gamma"].ap()
    oa_p = nc.kio["out"].ap().rearrange("(a p) w -> p a w", p=P)

    with ExitStack() as ctx:
        ep = ctx.enter_context

        p_misc = ep(tc.tile_pool(name="misc", bufs=1))
        p_nat = ep(tc.tile_pool(name="nat", bufs=cfg.get("nat_bufs", 3)))
        p_T = ep(tc.tile_pool(name="pT", bufs=1))
        p_att = ep(tc.tile_pool(name="att", bufs=CT))
        p_attT = ep(tc.tile_pool(name="attT", bufs=CT))
        p_small = ep(tc.tile_pool(name="small", bufs=2))
        p_vc = ep(tc.tile_pool(name="vc", bufs=cfg.get("vc_bufs", 3)))
        p_vb = ep(tc.tile_pool(name="vb", bufs=1))
        p_es = ep(tc.tile_pool(name="es", bufs=cfg.get("es_bufs", 3)))

        # gamma broadcast across partitions: [1,1] DRAM -> [128,1] SBUF
        g128 = p_misc.tile([P, 1], f32)
        nc.sync.dma_start(g128[:], ga.broadcast_to([P, 1]))

        ident = p_misc.tile([P, P], bf16)
        make_identity(nc, ident[:])

        # packed transposed tensors, one dedicated tile per n-group:
        # qT[g][p, ct, jj, cl] = q[ct*128+cl, g*GW + jj*128 + p]
        qT = [
            p_T.tile([P, CT, GJ, P], bf16, tag=f"qT{g}", name=f"qT{g}")
            for g in range(NG)
        ]
        kT = [
            p_T.tile([P, CT, GJ, P], bf16, tag=f"kT{g}", name=f"kT{g}")
            for g in range(NG)
        ]
        # v bf16 column blocks, dedicated: vb[vg][dt] is [128, VW]
        vb = [
            [
                p_vb.tile([P, VW], bf16, tag=f"vb{vg}{dat}", name=f"vb{vg}{dat}")
                for dat in range(CT)
            ]
            for vg in range(VG)
        ]

        att = []
        with tc.tile_pool(name="energy", bufs=CT, space="PSUM") as p_energy:
            e_ps = [
                p_energy.tile([P, 512], f32, tag="e", name=f"e{c}")
                for c in range(CT)
            ]

            # ---- q/k front: load -> cast -> xbar-transpose -> mm1 per group
            last_k_load = None
            if do("loads_qk"):
                for g in range(NG):
                    for t in range(CT):
                        qn32 = p_nat.tile(
                            [P, GW], f32, tag="qn32", name=f"qn32_{g}{t}"
                        )
                        nc.sync.dma_start(qn32[:], qa[t][:, ts(g, GW)])
                        qn = p_nat.tile([P, GW], bf16, tag="qn", name=f"qn{g}{t}")
                        nc.vector.tensor_copy(qn[:], qn32[:])
                        if do("tpose"):
                            nc.scalar.dma_start(
                                qT[g][:, t], qn[:], transpose=True
                            )
                        kn32 = p_nat.tile(
                            [P, GW], f32, tag="kn32", name=f"kn32_{g}{t}"
                        )
                        last_k_load = nc.gpsimd.dma_start(
                            kn32[:], ka[t][:, ts(g, GW)]
                        )
                        kn = p_nat.tile([P, GW], bf16, tag="kn", name=f"kn{g}{t}")
                        nc.vector.tensor_copy(kn[:], kn32[:])
                        if do("tpose"):
                            nc.scalar.dma_start(
                                kT[g][:, t], kn[:], transpose=True
                            )

                    if not (do("tpose") and do("mm1")):
                        continue
                    # mm1 for this group: e[ct] += qT[g][:,ct,jj,:].T @ kT-jj
                    for jj in range(GJ):
                        rhs = kT[g][:, :, jj, :]
                        for ct in range(CT):
                            nc.tensor.matmul(
                                e_ps[ct][:],
                                qT[g][:, ct, jj, :],
                                rhs,
                                start=(g == 0 and jj == 0),
                                stop=(g == NG - 1 and jj == GJ - 1),
                            )

            # ---- v loads (column blocks) + casts, ordered after k on the
            # SWDGE ring so they can't starve the mm1-critical k stream
            if do("loads_v"):
                from concourse.tile_rust import add_dep_helper

                for vg in range(VG):
                    for dat in range(CT):
                        vc = p_vc.tile(
                            [P, VW], f32, tag="vc", name=f"vc{vg}{dat}"
                        )
                        vload = nc.gpsimd.dma_start(vc[:], va[dat][:, ts(vg, VW)])
                        if vg == 0 and dat == 0 and last_k_load is not None:
                            add_dep_helper(
                                last_k_load.ins,
                                vload.ins,
                                sync=False,
                                reason="v loads after k loads on SWDGE ring",
                            )
                        if dat % 2 == 0:
                            nc.vector.tensor_copy(vb[vg][dat][:], vc[:])
                        else:
                            nc.scalar.copy(vb[vg][dat][:], vc[:])

            if not (do("loads_qk") and do("tpose") and do("mm1")):
                return

            if cfg.get("dump_energy"):
                oa = nc.kio["out"].ap().rearrange("(a p) w -> a p w", p=P)
                for ct in range(CT):
                    ed = p_es.tile([P, 512], f32, tag="ed", name=f"ed{ct}")
                    nc.vector.tensor_copy(ed[:], e_ps[ct][:])
                    nc.sync.dma_start(oa[ct][:, 0:512], ed[:])
                return

            # ---- softmax(-energy) rows; gamma/rowsum folded; +I diagonal
            for ct in range(CT):
                rowmin = p_small.tile([P, 1], f32)
                nc.vector.tensor_reduce(
                    rowmin[:], e_ps[ct][:], axis=X, op=mybir.AluOpType.min
                )
                pexp = p_att.tile([P, 512], bf16, tag="att", name=f"att{ct}")
                rowsum = p_small.tile([P, 1], f32)
                nc.scalar.activation(
                    pexp[:],
                    e_ps[ct][:],
                    mybir.ActivationFunctionType.Exp,
                    bias=rowmin[:, 0:1],
                    scale=-1.0,
                    accum_out=rowsum[:, 0:1],
                )
                recip = p_small.tile([P, 1], f32)
                nc.vector.reciprocal(recip[:], rowsum[:])
                srow = p_small.tile([P, 1], f32)
                nc.vector.tensor_scalar_mul(srow[:], recip[:], g128[:, 0:1])
                nc.vector.tensor_scalar_mul(pexp[:], pexp[:], srow[:, 0:1])
                # fold "+ V": att_eff = gamma*att + I  (diagonal block at ct)
                nc.vector.tensor_add(
                    pexp[:, ts(ct, P)], pexp[:, ts(ct, P)], ident[:]
                )
                att.append(pexp)

        if not do("mm2"):
            return

        # ---- transpose att (bf16) via PE into attT[dt][:, ct-block]
        attT = []
        with tc.tile_pool(name="pst", bufs=2, space="PSUM") as p_pst:
            for dat in range(CT):
                at = p_attT.tile([P, C], bf16, tag="attT", name=f"attT{dat}")
                for ct in range(CT):
                    pst = p_pst.tile([P, P], bf16)
                    nc.tensor.transpose(pst[:], att[ct][:, ts(dat, P)], ident[:])
                    if ct % 2 == 0:
                        nc.vector.tensor_copy(at[:, ts(ct, P)], pst[:])
                    else:
                        nc.scalar.copy(at[:, ts(ct, P)], pst[:])
                attT.append(at)

        # ---- mm2: psum = (gamma*att + I) @ V_bf16; epilogue = plain copy
        with tc.tile_pool(name="ps2", bufs=4, space="PSUM") as p_ps2:
            for no in range(NO):
                vg, hb = (no * 512) // VW, (no * 512) % VW
                es = p_es.tile([P, CT, 512], f32, tag="es", name=f"es{no}")
                for ct in range(CT):
                    ps2 = p_ps2.tile([P, 512], f32)
                    for dat in range(CT):
                        nc.tensor.matmul(
                            ps2[:],
                            attT[dat][:, ts(ct, P)],
                            vb[vg][dat][:, hb : hb + 512],
                            start=(dat == 0),
                            stop=(dat == CT - 1),
                        )
                    if ct % 2 == 0:
                        nc.vector.tensor_copy(es[:, ct, :], ps2[:])
                    else:
                        nc.scalar.copy(es[:, ct, :], ps2[:])
                nc.sync.dma_start(oa_p[:, :, ts(no, 512)], es[:])


def build(repeat=1, cfg=None, loop_n=None):
    import concourse.mybir as mybir
    import concourse.tile as tile
    from concourse import bacc

    dt = mybir.dt
    nc = bacc.Bacc("TRN2", target_bir_lowering=False, debug=False)
    nc.kio = {}
    for name in ("q", "k", "v"):
        nc.kio[name] = nc.dram_tensor(
            name, [C, N], dt.float32, kind="ExternalInput"
        )
    nc.kio["gamma"] = nc.dram_tensor(
        "gamma", [1, 1], dt.float32, kind="ExternalInput"
    )
    nc.kio["out"] = nc.dram_tensor(
        "out", [C, N], dt.float32, kind="ExternalOutput"
    )
    with tile.TileContext(nc) as tc:
        if loop_n is not None:
            with tc.For_i(0, loop_n, 1):
                _body(nc, tc, cfg)
        else:
            for _ in range(repeat):
                _body(nc, tc, cfg)
    nc.compile()
    return nc


def _get_nc():
    if "nc" not in _nc_cache:
        _nc_cache["nc"] = build(repeat=1)
    return _nc_cache["nc"]


def make_in_maps(q, k, v, gamma):
    q = np.ascontiguousarray(np.asarray(q, dtype=np.float32).reshape(B, C, N))
    k = np.ascontiguousarray(np.asarray(k, dtype=np.float32).reshape(B, C, N))
    v = np.ascontiguousarray(np.asarray(v, dtype=np.float32).reshape(B, C, N))
    g = np.asarray(gamma, dtype=np.float32).reshape(1, 1)
    return [
        {"q": q[i], "k": k[i], "v": v[i], "gamma": g} for i in range(B)
    ]


def kernel(q, k, v, gamma):
    from concourse import bass_utils

    nc = _get_nc()
    in_maps = make_in_maps(q, k, v, gamma)
    res = bass_utils.run_bass_kernel_spmd(nc, in_maps, core_ids=list(range(B)))
    out = np.stack([res.results[i]["out"] for i in range(B)])
    return out.reshape(B, C, H, W).astype(np.float32, copy=False)


# revision 28
# speedup vs baseline: 1.9342x; 1.9342x over previous
"""CAM (channel attention) kernel for Trainium2, 8-core data-parallel over batch.

Per batch item (one per NeuronCore):
    energy = Q @ K^T  (C x C, contract over N=4096)
    att    = softmax(max(energy) - energy) = softmax(-energy)
    out    = gamma * (att @ V) + V  computed as (gamma*att + I) @ V_bf16

Pipeline (per core; q,k,v: [C=512, N=4096] f32 in DRAM):
  - q chunks [128,1024] f32 stream on the SP HWDGE ring, k chunks on the
    gpsimd SWDGE ring; DVE casts q to bf16, ACT casts k; the scalar HWDGE
    ring xbar-transposes both into packed qT/kT tiles, one tile per n-group,
    laid out [p, ctile, jj, 128] so every xbar dst is contiguous per
    partition (fast path) and mm1 reads qT contiguously / kT via 3D AP.
  - mm1 accumulates energy in 4 PSUM banks [128c, 512d] over the 32 n-chunks,
    group by group as transposes land.
  - softmax over the free dim: DVE min, ACT exp(bias=rowmin, scale=-1) with
    fused row-sum, DVE reciprocal; gamma/rowsum folded into att scaling and
    identity added on the diagonal so mm2 directly yields gamma*(att@V)+V.
  - att transposed via PE (identity matmul) into attT (bf16).
  - v loaded as column-blocks [128,1024] f32 (SWDGE, after k), cast to bf16;
    mm2 does 4 accumulating matmuls per (ctile, 512-col chunk); epilogue is a
    plain PSUM->SBUF copy (DVE/ACT alternating); stores on the SP ring.
"""

import numpy as np

B, C, H, W = 8, 512, 64, 64
N = H * W  # 4096
P = 128
CT = C // P  # 4 c-tiles
NJ = N // P  # 32 n-chunks
NO = N // 512  # 8 output column chunks

_nc_cache: dict = {}


def _body(nc, tc, cfg):
    from contextlib import ExitStack

    import concourse.mybir as mybir
    from concourse.bass import ts
    from concourse.masks import make_identity

    cfg = cfg or {}

    dt = mybir.dt
    f32, bf16 = dt.float32, dt.bfloat16
    X = mybir.AxisListType.X

    NG = cfg.get("ng", 4)  # n-groups for the q/k front
    GJ = NJ // NG  # 128-chunks per group
    GW = GJ * P  # columns per group chunk (1024)
    VG = cfg.get("vg", 4)  # v column-groups
    VW = N // VG  # 1024
    do = lambda phase: phase not in cfg.get("skip", ())

    qa = nc.kio["q"].ap().rearrange("(a p) w -> a p w", p=P)
    ka = nc.kio["k"].ap().rearrange("(a p) w -> a p w", p=P)
    va = nc.kio["v"].ap().rearrange("(a p) w -> a p w", p=P)
    ga = nc.kio["gamma"].ap()
    oa_p = nc.kio["out"].ap().rearrange("(a p) w -> p a w", p=P)

    with ExitStack() as ctx:
        ep = ctx.enter_context

        p_misc = ep(tc.tile_pool(name="misc", bufs=1))
        p_nat = ep(tc.tile_pool(name="nat", bufs=cfg.get("nat_bufs", 3)))
        p_T = ep(tc.tile_pool(name="pT", bufs=1))
        p_att = ep(tc.tile_pool(name="att", bufs=CT))
        p_attT = ep(tc.tile_pool(name="attT", bufs=CT))
        p_small = ep(tc.tile_pool(name="small", bufs=2))
        p_vc = ep(tc.tile_pool(name="vc", bufs=cfg.get("vc_bufs", 3)))
        p_vb = ep(tc.tile_pool(name="vb", bufs=1))
        p_es = ep(tc.tile_pool(name="es", bufs=cfg.get("es_bufs", 3)))

        # gamma broadcast across partitions: [1,1] DRAM -> [128,1] SBUF
        g128 = p_misc.tile([P, 1], f32)
        nc.sync.dma_start(g128[:], ga.broadcast_to([P, 1]))

        ident = p_misc.tile([P, P], bf16)
        make_identity(nc, ident[:])
        f32t = cfg.get("f32t", False)  # PE-transpose f32 staging directly
        if f32t:
            ident32 = p_misc.tile([P, P], f32)
            make_identity(nc, ident32[:])

        # packed transposed tensors, one dedicated tile per n-group:
        # qT[g][p, ct, jj, cl] = q[ct*128+cl, g*GW + jj*128 + p]
        qT = [
            p_T.tile([P, CT, GJ, P], bf16, tag=f"qT{g}", name=f"qT{g}")
            for g in range(NG)
        ]
        kT = [
            p_T.tile([P, CT, GJ, P], bf16, tag=f"kT{g}", name=f"kT{g}")
            for g in range(NG)
        ]
        tq = cfg.get("tq", "xbar")  # q transpose path: xbar | pe
        tk = cfg.get("tk", "xbar")  # k transpose path: xbar | pe
        engs = {
            "sync": nc.sync,
            "scalar": nc.scalar,
            "gpsimd": nc.gpsimd,
        }
        k_eng = engs[cfg.get("k_eng", "gpsimd")]
        v_eng = engs[cfg.get("v_eng", "gpsimd")]
        store_split = cfg.get("store_split", False)
        # v bf16 column blocks, dedicated: vb[vg][dt] is [128, VW]
        vb = [
            [
                p_vb.tile([P, VW], bf16, tag=f"vb{vg}{dat}", name=f"vb{vg}{dat}")
                for dat in range(CT)
            ]
            for vg in range(VG)
        ]

        if cfg.get("rowprobe"):
            # pure-load probe: q,k,v as [128, 4096] row tiles (16KB runs)
            p_row = ctx.enter_context(
                tc.tile_pool(name="row", bufs=cfg.get("row_bufs", 2))
            )
            for src in (qa, ka, va):
                for t in range(CT):
                    r = p_row.tile([P, N], f32, tag="row")
                    nc.sync.dma_start(r[:], src[t])
                    rsum = p_small.tile([P, 1], f32)
                    nc.vector.tensor_reduce(
                        rsum[:],
                        r[:, 0:512],
                        axis=mybir.AxisListType.X,
                        op=mybir.AluOpType.add,
                    )
            return

        att = []
        with tc.tile_pool(name="energy", bufs=CT, space="PSUM") as p_energy:
            e_ps = [
                p_energy.tile([P, 512], f32, tag="e", name=f"e{c}")
                for c in range(CT)
            ]

            # ---- q/k front: load -> cast -> transpose -> mm1 per group
            last_k_load = None
            if do("loads_qk"):
                with ExitStack() as fctx:
                    if tq == "pe" or tk == "pe":
                        p_tp = fctx.enter_context(
                            tc.tile_pool(name="tp", bufs=4, space="PSUM")
                        )

                    def pe_transpose(dst, src, par):
                        # dst = [P, GJ, P] slice of qT/kT; src = [P, GW]
                        f32src = src.dtype == f32
                        tdt = f32 if f32src else bf16
                        tid = ident32 if f32src else ident
                        nb = 2 if f32src else 4  # blocks per PSUM tile (bank)
                        for h in range(GJ // nb):
                            tp = p_tp.tile([P, nb, P], tdt, tag="tp")
                            for j4 in range(nb):
                                nc.tensor.transpose(
                                    tp[:, j4, :],
                                    src[:, ts(h * nb + j4, P)],
                                    tid[:],
                                )
                            if (h + par) % 2 == 0:
                                nc.vector.tensor_copy(dst[:, ts(h, nb)], tp[:])
                            else:
                                nc.scalar.copy(dst[:, ts(h, nb)], tp[:])
                    for g in range(NG):
                        for t in range(CT):
                            qn32 = p_nat.tile(
                                [P, GW], f32, tag="qn32", name=f"qn32_{g}{t}"
                            )
                            nc.sync.dma_start(qn32[:], qa[t][:, ts(g, GW)])
                            if f32t and tq == "pe":
                                if do("tpose"):
                                    pe_transpose(qT[g][:, t], qn32, 0)
                            else:
                                qn = p_nat.tile(
                                    [P, GW], bf16, tag="qn", name=f"qn{g}{t}"
                                )
                                nc.vector.tensor_copy(qn[:], qn32[:])
                                if do("tpose"):
                                    if tq == "pe":
                                        pe_transpose(qT[g][:, t], qn, 0)
                                    else:
                                        nc.scalar.dma_start(
                                            qT[g][:, t], qn[:], transpose=True
                                        )
                            kn32 = p_nat.tile(
                                [P, GW], f32, tag="kn32", name=f"kn32_{g}{t}"
                            )
                            last_k_load = k_eng.dma_start(
                                kn32[:], ka[t][:, ts(g, GW)]
                            )
                            if f32t and tk == "pe":
                                if do("tpose"):
                                    pe_transpose(kT[g][:, t], kn32, 1)
                            else:
                                kn = p_nat.tile(
                                    [P, GW], bf16, tag="kn", name=f"kn{g}{t}"
                                )
                                nc.vector.tensor_copy(kn[:], kn32[:])
                                if do("tpose"):
                                    if tk == "pe":
                                        pe_transpose(kT[g][:, t], kn, 1)
                                    else:
                                        nc.scalar.dma_start(
                                            kT[g][:, t], kn[:], transpose=True
                                        )

                        if not (do("tpose") and do("mm1")):
                            continue
                        # mm1 for group g: e[ct] += qT[g][:,ct,jj,:].T @ kT-jj
                        # last group runs ct-outer so e_ps[0] completes early
                        # and softmax/attT/mm2 can pipeline behind the tail
                        if g == NG - 1:
                            for ct in range(CT):
                                for jj in range(GJ):
                                    nc.tensor.matmul(
                                        e_ps[ct][:],
                                        qT[g][:, ct, jj, :],
                                        kT[g][:, :, jj, :],
                                        start=(g == 0 and jj == 0),
                                        stop=(jj == GJ - 1),
                                    )
                        else:
                            for jj in range(GJ):
                                rhs = kT[g][:, :, jj, :]
                                for ct in range(CT):
                                    nc.tensor.matmul(
                                        e_ps[ct][:],
                                        qT[g][:, ct, jj, :],
                                        rhs,
                                        start=(g == 0 and jj == 0),
                                        stop=False,
                                    )

            # ---- v loads (column blocks) + casts, ordered after k on the
            # SWDGE ring so they can't starve the mm1-critical k stream
            if do("loads_v"):
                from concourse.tile_rust import add_dep_helper

                for vg in range(VG):
                    for dat in range(CT):
                        vc = p_vc.tile(
                            [P, VW], f32, tag="vc", name=f"vc{vg}{dat}"
                        )
                        vload = v_eng.dma_start(vc[:], va[dat][:, ts(vg, VW)])
                        if vg == 0 and dat == 0 and last_k_load is not None:
                            # keep HBM bandwidth for the mm1-critical q/k
                            # stream: v transfers begin after the k stream
                            add_dep_helper(
                                last_k_load.ins,
                                vload.ins,
                                sync=bool(cfg.get("v_delay_sync", v_eng is not k_eng)),
                                reason="v loads after k loads",
                            )
                        if dat % 2 == 0:
                            nc.vector.tensor_copy(vb[vg][dat][:], vc[:])
                        else:
                            nc.scalar.copy(vb[vg][dat][:], vc[:])

            if not (do("loads_qk") and do("tpose") and do("mm1")):
                return

            if cfg.get("dump_energy"):
                oa = nc.kio["out"].ap().rearrange("(a p) w -> a p w", p=P)
                for ct in range(CT):
                    ed = p_es.tile([P, 512], f32, tag="ed", name=f"ed{ct}")
                    nc.vector.tensor_copy(ed[:], e_ps[ct][:])
                    nc.sync.dma_start(oa[ct][:, 0:512], ed[:])
                return

            # ---- softmax(-energy) rows; gamma/rowsum folded; +I diagonal
            for ct in range(CT):
                rowmin = p_small.tile([P, 1], f32)
                nc.vector.tensor_reduce(
                    rowmin[:], e_ps[ct][:], axis=X, op=mybir.AluOpType.min
                )
                pexp = p_att.tile([P, 512], bf16, tag="att", name=f"att{ct}")
                rowsum = p_small.tile([P, 1], f32)
                nc.scalar.activation(
                    pexp[:],
                    e_ps[ct][:],
                    mybir.ActivationFunctionType.Exp,
                    bias=rowmin[:, 0:1],
                    scale=-1.0,
                    accum_out=rowsum[:, 0:1],
                )
                recip = p_small.tile([P, 1], f32)
                nc.vector.reciprocal(recip[:], rowsum[:])
                srow = p_small.tile([P, 1], f32)
                nc.vector.tensor_scalar_mul(srow[:], recip[:], g128[:, 0:1])
                nc.vector.tensor_scalar_mul(pexp[:], pexp[:], srow[:, 0:1])
                # fold "+ V": att_eff = gamma*att + I  (diagonal block at ct)
                nc.vector.tensor_add(
                    pexp[:, ts(ct, P)], pexp[:, ts(ct, P)], ident[:]
                )
                att.append(pexp)

        if not do("mm2"):
            return

        # ---- transpose att (bf16) via PE into attT[dt][:, ct-block];
        # ct-outer so mm2's first (ct=0) chain unblocks after 4 transposes
        attT = [
            p_attT.tile([P, C], bf16, tag="attT", name=f"attT{dat}")
            for dat in range(CT)
        ]
        with tc.tile_pool(name="pst", bufs=2, space="PSUM") as p_pst:
            for ct in range(CT):
                for dat in range(CT):
                    pst = p_pst.tile([P, P], bf16)
                    nc.tensor.transpose(pst[:], att[ct][:, ts(dat, P)], ident[:])
                    if dat % 2 == 0:
                        nc.vector.tensor_copy(attT[dat][:, ts(ct, P)], pst[:])
                    else:
                        nc.scalar.copy(attT[dat][:, ts(ct, P)], pst[:])

        # ---- mm2: psum = (gamma*att + I) @ V_bf16; epilogue = plain copy
        with tc.tile_pool(name="ps2", bufs=4, space="PSUM") as p_ps2:
            for no in range(NO):
                vg, hb = (no * 512) // VW, (no * 512) % VW
                es = p_es.tile([P, CT, 512], f32, tag="es", name=f"es{no}")
                for ct in range(CT):
                    ps2 = p_ps2.tile([P, 512], f32)
                    for dat in range(CT):
                        nc.tensor.matmul(
                            ps2[:],
                            attT[dat][:, ts(ct, P)],
                            vb[vg][dat][:, hb : hb + 512],
                            start=(dat == 0),
                            stop=(dat == CT - 1),
                        )
                    if ct % 2 == 0:
                        nc.vector.tensor_copy(es[:, ct, :], ps2[:])
                    else:
                        nc.scalar.copy(es[:, ct, :], ps2[:])
                s_eng = nc.scalar if (store_split and no % 2) else nc.sync
                s_eng.dma_start(oa_p[:, :, ts(no, 512)], es[:])


BEST_CFG = {
    "tq": "pe",
    "tk": "pe",
    "k_eng": "sync",
    "v_eng": "sync",
    "ng": 8,
    "vg": 2,
    "vc_bufs": 2,
    "nat_bufs": 6,
    "es_bufs": 2,
    "store_split": True,
}


def build(repeat=1, cfg=None, loop_n=None):
    import concourse.mybir as mybir
    import concourse.tile as tile
    from concourse import bacc

    if cfg is None:
        cfg = dict(BEST_CFG)
    dt = mybir.dt
    nc = bacc.Bacc("TRN2", target_bir_lowering=False, debug=False)
    nc.kio = {}
    for name in ("q", "k", "v"):
        nc.kio[name] = nc.dram_tensor(
            name, [C, N], dt.float32, kind="ExternalInput"
        )
    nc.kio["gamma"] = nc.dram_tensor(
        "gamma", [1, 1], dt.float32, kind="ExternalInput"
    )
    nc.kio["out"] = nc.dram_tensor(
        "out", [C, N], dt.float32, kind="ExternalOutput"
    )
    with tile.TileContext(nc) as tc:
        if loop_n is not None:
            hints = ()
            if (cfg or {}).get("hint"):
                hints = (
                    mybir.EngineType.PE,
                    mybir.EngineType.DVE,
                    mybir.EngineType.Activation,
                )
            stag = bool((cfg or {}).get("stag"))
            with tc.For_i(0, loop_n, 1, hint_engines=hints, staggered_reset=stag):
                _body(nc, tc, cfg)
        else:
            for _ in range(repeat):
                _body(nc, tc, cfg)
    nc.compile()
    return nc


def _get_nc():
    if "nc" not in _nc_cache:
        _nc_cache["nc"] = build(repeat=1)
    return _nc_cache["nc"]


def make_in_maps(q, k, v, gamma):
    q = np.ascontiguousarray(np.asarray(q, dtype=np.float32).reshape(B, C, N))
    k = np.ascontiguousarray(np.asarray(k, dtype=np.float32).reshape(B, C, N))
    v = np.ascontiguousarray(np.asarray(v, dtype=np.float32).reshape(B, C, N))
    g = np.asarray(gamma, dtype=np.float32).reshape(1, 1)
    return [
        {"q": q[i], "k": k[i], "v": v[i], "gamma": g} for i in range(B)
    ]


def kernel(q, k, v, gamma):
    from concourse import bass_utils

    nc = _get_nc()
    in_maps = make_in_maps(q, k, v, gamma)
    res = bass_utils.run_bass_kernel_spmd(nc, in_maps, core_ids=list(range(B)))
    out = np.stack([res.results[i]["out"] for i in range(B)])
    return out.reshape(B, C, H, W).astype(np.float32, copy=False)
